# revision 28
# baseline (speedup 1.0000x reference)
"""Trainium2 Bass kernel for a 2-layer GCN (nn_Net_49065706389774) — v2.

out = (S relu(S x W1 + b1)) (W2 WL) + (b2 WL + bL),  S = D^-1/2 (A+I) D^-1/2

v2 changes vs baseline:
 - node ownership n % 8 == c (so quarters of local rows = contiguous global
   node ranges) enabling 4 pipelined AllGathers overlapped with conv compute;
 - conv2 aggregates g = relu(h) @ (W2 WL) (16 features) instead of h (64);
 - conv2 is chunk-major with all 98 block accumulators live in hand-packed
   PSUM banks; epilogue2 is matmul-free;
 - epilogue1 computes h transposed (one PE transpose) and reuses relu(h).T
   as lhsT for the Wf matmul.
"""
import numpy as np
import ml_dtypes

import concourse.bass as bass
import concourse.bacc as bacc
import concourse.mybir as mybir
import concourse.tile as tile
from concourse import bass_utils

F32 = mybir.dt.float32
BF16 = mybir.dt.bfloat16
I16 = mybir.dt.int16
AF = mybir.ActivationFunctionType
ALU = mybir.AluOpType


def make_dims(N=100000):
    NC = 8
    N_LOC = N // NC          # 12500
    BLK = 128
    GRP = 4
    N_BLK = (N_LOC + BLK - 1) // BLK      # 98
    N_GRP = (N_BLK + GRP - 1) // GRP      # 25
    GQ = [0, 8, 14, 20, 25]               # group bounds per quarter
    RB = [min(g * GRP, N_BLK) for g in GQ]            # block bounds [0,30,54,78,98]
    R = [min(rb * BLK, N_LOC) for rb in RB]           # local row bounds
    QN = [NC * r for r in R]                          # global node bounds
    SQ = [R[j + 1] - R[j] for j in range(4)]
    return dict(N=N, NC=NC, N_LOC=N_LOC, BLK=BLK, GRP=GRP, N_BLK=N_BLK,
                N_GRP=N_GRP, GQ=GQ, RB=RB, R=R, QN=QN, SQ=SQ,
                F_IN=16, H1=64, COLS_PER_CALL=8)


def _make_schedule(core_segs, NC, N_BLK, COLS_PER_CALL, seg_group=None,
                   grp=4, per_segment_flags=False):
    """core_segs[sg][c] = (idx16 int64 array, kd int64 array) sorted by kd.

    Returns schedule + per-core idx/dstloc streams + flags.
    seg_group: optional list mapping segment -> dst group (for conv1 dummy-job
    placement which must stay within the block's group); None = any segment.
    per_segment_flags: start/stop accumulation scopes are per segment (conv2's
    short-lived psum accumulators) instead of global.
    """
    BLK = 128
    n_seg = len(core_segs)
    seg_cols = []
    for sg in range(n_seg):
        mx = max(len(core_segs[sg][c][0]) for c in range(NC))
        seg_cols.append((mx + BLK - 1) // BLK)

    schedule = []
    core_idx16 = [[] for _ in range(NC)]
    core_dst_of_slot = [[] for _ in range(NC)]
    core_blk_of_slot = [[] for _ in range(NC)]
    for sg in range(n_seg):
        C = seg_cols[sg]
        if C == 0:
            schedule.append(dict(sg=sg, cols=0, calls=[], jobs=[]))
            continue
        nslots = C * BLK
        col_jobs = [set() for _ in range(C)]
        for c in range(NC):
            iv, kdv = core_segs[sg][c]
            k = len(iv)
            i16 = np.zeros(nslots, np.int16)
            i16[:k] = iv.astype(np.int16)
            dl = np.full(nslots, -1, np.int32)
            dl[:k] = kdv
            bl = np.full(nslots, -1, np.int32)
            bl[:k] = kdv // BLK
            core_idx16[c].append(i16)
            core_dst_of_slot[c].append(dl)
            core_blk_of_slot[c].append(bl)
            for col in range(C):
                for b in np.unique(bl[col * BLK:(col + 1) * BLK]):
                    if b >= 0:
                        col_jobs[col].add(int(b))
        prev = None
        for col in range(C):
            if not col_jobs[col]:
                fallback = prev if prev is not None else (
                    seg_group[sg] * grp if seg_group is not None else 0)
                col_jobs[col] = {fallback}
            prev = max(col_jobs[col])
        calls = []
        off = 0
        while off < C:
            calls.append(min(COLS_PER_CALL, C - off))
            off += COLS_PER_CALL
        schedule.append(dict(sg=sg, cols=C, calls=calls,
                             jobs=[sorted(col_jobs[col]) for col in range(C)]))

    # blocks with zero jobs -> inject dummy job so psum gets start/stop
    jobs_per_block = np.zeros(N_BLK, np.int64)
    for seg in schedule:
        for jl in seg["jobs"]:
            for b in jl:
                jobs_per_block[b] += 1
    for b in range(N_BLK):
        if jobs_per_block[b] == 0:
            placed = False
            for si, seg in enumerate(schedule):
                if seg["cols"] == 0:
                    continue
                if seg_group is not None and seg_group[si] != b // grp:
                    continue
                seg["jobs"][0] = sorted(set(seg["jobs"][0]) | {b})
                jobs_per_block[b] += 1
                placed = True
                break
            assert placed, f"no segment for dummy job of block {b}"

    # start/stop flags in traversal order
    flags = []
    if per_segment_flags:
        # Accumulation scopes are per (block, run of consecutive columns)
        # within a segment: short-lived psum tiles drained at each run end.
        for seg in schedule:
            occ = {}
            for col in range(seg["cols"]):
                for b in seg["jobs"][col]:
                    occ.setdefault(b, []).append(col)
            rs, re = {}, {}
            for b, cols in occ.items():
                for i, col in enumerate(cols):
                    rs[(b, col)] = (i == 0) or (cols[i - 1] != col - 1)
                    re[(b, col)] = (i == len(cols) - 1) or (cols[i + 1] != col + 1)
            for col in range(seg["cols"]):
                for b in seg["jobs"][col]:
                    flags.append((rs[(b, col)], re[(b, col)]))
        # pool-size check: max concurrently-open accumulators in emission order
        open_b, max_open = set(), 0
        ji = 0
        for seg in schedule:
            for col in range(seg["cols"]):
                for b in seg["jobs"][col]:
                    st, sp = flags[ji]
                    if st:
                        open_b.add(b)
                        max_open = max(max_open, len(open_b))
                    if sp:
                        open_b.discard(b)
                    ji += 1
        assert max_open <= 3, f"psum acc pool too small: {max_open} open"
        n_jobs = len(flags)
    else:
        first_seen, last_seen = {}, {}
        ji = 0
        for seg in schedule:
            for col in range(seg["cols"]):
                for b in seg["jobs"][col]:
                    if b not in first_seen:
                        first_seen[b] = ji
                    last_seen[b] = ji
                    ji += 1
        n_jobs = ji
        ji = 0
        for seg in schedule:
            for col in range(seg["cols"]):
                for b in seg["jobs"][col]:
                    flags.append((ji == first_seen[b], ji == last_seen[b]))
                    ji += 1

    # per-core streams
    per_core = []
    for c in range(NC):
        idx16 = (np.concatenate(core_idx16[c]) if core_idx16[c]
                 else np.zeros(0, np.int16))
        S = len(idx16)
        assert S % 16 == 0
        idx_w = np.tile(idx16.reshape(S // 16, 16).T, (8, 1))  # [128, S/16]
        dstlocs = []
        seg_i = 0
        for seg in schedule:
            if seg["cols"] == 0:
                continue
            dl = core_dst_of_slot[c][seg_i]
            bl = core_blk_of_slot[c][seg_i]
            for col in range(seg["cols"]):
                dcol = dl[col * BLK:(col + 1) * BLK]
                bcol = bl[col * BLK:(col + 1) * BLK]
                for b in seg["jobs"][col]:
                    rel = np.where(bcol == b, dcol - b * BLK, -1).astype(np.float32)
                    dstlocs.append(rel)
            seg_i += 1
        dstloc = np.stack(dstlocs, axis=1)  # [128, n_jobs]
        assert dstloc.shape[1] == n_jobs
        per_core.append(dict(idx_w=idx_w, dstloc=dstloc))

    max_jobs_per_call = 0
    for seg in schedule:
        off = 0
        for ncols in seg["calls"]:
            j = sum(len(seg["jobs"][off + k]) for k in range(ncols))
            max_jobs_per_call = max(max_jobs_per_call, j)
            off += ncols

    n_slots = sum(s["cols"] for s in schedule) * BLK
    # slot column offsets for gather idx addressing
    sc = 0
    for seg in schedule:
        seg["_slot_col0"] = sc
        sc += seg["cols"]
    return dict(schedule=schedule, per_core=per_core, flags=flags,
                n_jobs=n_jobs, n_slots=n_slots,
                max_jobs_per_call=max_jobs_per_call)


def preprocess(edge_index, dims):
    N, NC, BLK, GRP = dims["N"], dims["NC"], dims["BLK"], dims["GRP"]
    N_BLK, N_GRP = dims["N_BLK"], dims["N_GRP"]
    R, QN, SQ = dims["R"], dims["QN"], dims["SQ"]
    src = np.asarray(edge_index[0], np.int64)
    dst = np.asarray(edge_index[1], np.int64)
    deg = (np.bincount(dst, minlength=N) + 1.0).astype(np.float32)

    QNa = np.asarray(QN, np.int64)
    segs1 = [[None] * NC for _ in range(N_GRP * 4)]
    segs2 = [[None] * NC for _ in range(4)]
    for c in range(NC):
        m = (dst % NC) == c
        s = src[m]
        kd = dst[m] // NC
        qk = np.searchsorted(QNa, s, side="right") - 1
        # conv1: by (group, chunk, dst)
        gk = kd // (GRP * BLK)
        o1 = np.lexsort((kd, qk, gk))
        s1, kd1, q1, g1 = s[o1], kd[o1], qk[o1], gk[o1]
        for g in range(N_GRP):
            for q in range(4):
                mm = (g1 == g) & (q1 == q)
                segs1[g * 4 + q][c] = (s1[mm] - QN[q], kd1[mm])
        # conv2: by (chunk, dst)
        o2 = np.lexsort((kd, qk))
        s2, kd2, q2 = s[o2], kd[o2], qk[o2]
        for q in range(4):
            mm = q2 == q
            sq = s2[mm]
            idx2 = (sq % NC) * SQ[q] + (sq // NC - R[q])
            segs2[q][c] = (idx2, kd2[mm])

    seg_group1 = [sg // 4 for sg in range(N_GRP * 4)]
    sch1 = _make_schedule(segs1, NC, N_BLK, dims["COLS_PER_CALL"], seg_group1,
                          grp=GRP)
    sch2 = _make_schedule(segs2, NC, N_BLK, dims["COLS_PER_CALL"], None,
                          grp=GRP, per_segment_flags=True)
    return dict(deg=deg, sch1=sch1, sch2=sch2,
                jmax=max(sch1["max_jobs_per_call"], sch2["max_jobs_per_call"]))


def build(prep, dims):
    N, NC, N_LOC, BLK = dims["N"], dims["NC"], dims["N_LOC"], dims["BLK"]
    GRP, N_BLK, N_GRP = dims["GRP"], dims["N_BLK"], dims["N_GRP"]
    GQ, R, QN, SQ = dims["GQ"], dims["R"], dims["QN"], dims["SQ"]
    F_IN, H1 = dims["F_IN"], dims["H1"]
    JMAX = prep["jmax"]
    sch1, sch2 = prep["sch1"], prep["sch2"]

    S1 = sch1["per_core"][0]["idx_w"].shape[1]
    S2 = sch2["per_core"][0]["idx_w"].shape[1]
    J1 = sch1["n_jobs"]
    J2 = sch2["n_jobs"]

    nc = bacc.Bacc("TRN2", target_bir_lowering=False, debug=False,
                   num_devices=NC, num_swdge_queues=4)
    xt_t = nc.dram_tensor("xt", [N, 128], BF16, kind="ExternalInput").ap()
    idx1_t = nc.dram_tensor("idx1", [128, S1], I16, kind="ExternalInput").ap()
    dst1_t = nc.dram_tensor("dst1", [128, J1], BF16, kind="ExternalInput").ap()
    idx2_t = nc.dram_tensor("idx2", [128, S2], I16, kind="ExternalInput").ap()
    dst2_t = nc.dram_tensor("dst2", [128, J2], BF16, kind="ExternalInput").ap()
    dinv_t = nc.dram_tensor("dinv_blk", [128, N_BLK], F32, kind="ExternalInput").ap()
    ownx_t = nc.dram_tensor("ownx", [128, N_BLK * F_IN], F32, kind="ExternalInput").ap()
    w1b_t = nc.dram_tensor("w1b", [F_IN + 1, H1], F32, kind="ExternalInput").ap()
    wf_t = nc.dram_tensor("wf", [H1, 16], F32, kind="ExternalInput").ap()
    bf_t = nc.dram_tensor("bft", [128, 16], F32, kind="ExternalInput").ap()
    iota_t = nc.dram_tensor("iota", [128, JMAX * 128], BF16, kind="ExternalInput").ap()
    ident_t = nc.dram_tensor("ident", [128, 128], F32, kind="ExternalInput").ap()
    ones_t = nc.dram_tensor("ones", [1, 128], F32, kind="ExternalInput").ap()
    out_t = nc.dram_tensor("out", [N_LOC, 16], F32, kind="ExternalOutput").ap()

    with tile.TileContext(nc) as tc:
        with (
            tc.tile_pool(name="const", bufs=1) as constp,
            tc.tile_pool(name="dram", bufs=1, space="DRAM") as dramp,
            tc.tile_pool(name="gat", bufs=8) as gatp,
            tc.tile_pool(name="msk", bufs=4) as mskp,
            tc.tile_pool(name="acc1", bufs=4, space="PSUM") as accp1,
            tc.tile_pool(name="acc2", bufs=3, space="PSUM") as accp2,
            tc.tile_pool(name="pb", bufs=1, space="PSUM") as pbank,
            tc.tile_pool(name="sb", bufs=4) as sbp,
        ):
            idx1_sb = constp.tile([128, S1], I16)
            nc.sync.dma_start(out=idx1_sb[:], in_=idx1_t[:])
            dst1_sb = constp.tile([128, J1], BF16)
            nc.sync.dma_start(out=dst1_sb[:], in_=dst1_t[:])
            idx2_sb = constp.tile([128, S2], I16)
            nc.sync.dma_start(out=idx2_sb[:], in_=idx2_t[:])
            dst2_sb = constp.tile([128, J2], BF16)
            nc.sync.dma_start(out=dst2_sb[:], in_=dst2_t[:])
            dinv_sb = constp.tile([128, N_BLK], F32)
            nc.sync.dma_start(out=dinv_sb[:], in_=dinv_t[:])
            ownx_sb = constp.tile([128, N_BLK * F_IN], F32)
            nc.sync.dma_start(out=ownx_sb[:], in_=ownx_t[:])
            w1b_sb = constp.tile([F_IN + 1, H1], F32)
            nc.sync.dma_start(out=w1b_sb[:], in_=w1b_t[:])
            wf_sb = constp.tile([H1, 16], F32)
            nc.sync.dma_start(out=wf_sb[:], in_=wf_t[:])
            bf_sb = constp.tile([128, 16], F32)
            nc.sync.dma_start(out=bf_sb[:], in_=bf_t[:])
            iota_sb = constp.tile([128, JMAX * 128], BF16)
            nc.sync.dma_start(out=iota_sb[:], in_=iota_t[:])
            ident_sb = constp.tile([128, 128], F32)
            nc.sync.dma_start(out=ident_sb[:], in_=ident_t[:])
            own_g = constp.tile([128, N_BLK * 16], BF16)
            z1T_tiles = [constp.tile([F_IN + 1, 128], F32, name=f"z1T_{i}")
                         for i in range(2)]
            for t in z1T_tiles:
                nc.sync.dma_start(out=t[F_IN : F_IN + 1, :], in_=ones_t[:])

            cc_in = dramp.tile([N_LOC, 128], BF16)
            cc_outs = [dramp.tile([NC * SQ[j], 128], BF16, addr_space="Shared",
                                  name=f"cc_out{j}") for j in range(4)]

            # PSUM: conv1 acc pool 4 banks, conv2 acc pool 3 banks, epi1
            # 1 packed bank (tp/hpT/g share it; start=True clears a whole
            # bank's has_written, so each live accumulator owns a bank).
            epib = [pbank.tile([128, 272], F32, name="epi_0")]
            # conv2 block partial sums across chunks accumulate in SBUF
            sacc2 = constp.tile([128, N_BLK * 16], F32)

            gq = [0]

            def gather_call(table_ap, idx_sb, w0, ncols):
                nidx = ncols * BLK
                g_tile = gatp.tile([128, 8 * 128], BF16, tag="gat")
                nc.gpsimd.dma_gather(
                    out_ap=g_tile[:, : ncols * 128].rearrange("p (c e) -> p c e", e=128),
                    in_ap=table_ap,
                    idxs_ap=idx_sb[:, w0 : w0 + ncols * 8],
                    num_idxs=nidx,
                    num_idxs_reg=nidx,
                    elem_size=128,
                    queue_num=3,
                )
                gq[0] += 1
                return g_tile

            def mask_call(dst_sb, j0, J):
                mask_tile = mskp.tile([128, JMAX * 128], BF16, tag="msk")
                nc.vector.tensor_tensor(
                    out=mask_tile[:, : J * 128],
                    in0=dst_sb[:, j0 : j0 + J].to_broadcast((128, J, 128)),
                    in1=iota_sb[:, : J * 128],
                    op=ALU.is_equal,
                )
                return mask_tile

            def rows_of(b):
                return min(BLK, N_LOC - b * BLK)

            def epi1(b, acc_ap):
                r = rows_of(b)
                eb = epib[0]
                tp = eb[0:F_IN, 0:128]
                hpT = eb[0:H1, 128:256]
                gps = eb[:, 256:272]
                zs = sbp.tile([128, F_IN], F32, tag="zs1")
                nc.vector.tensor_tensor(out=zs[:], in0=acc_ap,
                                        in1=ownx_sb[:, b * F_IN:(b + 1) * F_IN],
                                        op=ALU.add)
                z1 = sbp.tile([128, F_IN], F32, tag="z1")
                nc.scalar.activation(z1[:], zs[:], AF.Copy, scale=dinv_sb[:, b : b + 1])
                nc.tensor.transpose(tp, z1[:], ident_sb[:])
                z1T = z1T_tiles[b % 2]
                nc.scalar.activation(z1T[:F_IN, :], tp, AF.Copy)
                nc.tensor.matmul(hpT, lhsT=w1b_sb[:], rhs=z1T[:], start=True, stop=True)
                rhT = sbp.tile([H1, 128], F32, tag="rhT")
                nc.scalar.activation(rhT[:], hpT, AF.Relu)
                nc.tensor.matmul(gps, lhsT=rhT[:], rhs=wf_sb[:], start=True, stop=True)
                gp = sbp.tile([128, 128], BF16, tag="gp")
                nc.scalar.activation(gp[:, 0:16], gps, AF.Copy, scale=dinv_sb[:, b : b + 1])
                nc.scalar.activation(own_g[:, b * 16:(b + 1) * 16], gp[:, 0:16], AF.Copy)
                nc.sync.dma_start(out=cc_in[b * BLK : b * BLK + r, :], in_=gp[:r, :])

            # ---------------- conv1 ----------------
            flags1 = sch1["flags"]
            job_i = [0]
            for g in range(N_GRP):
                blocks = list(range(g * GRP, min(g * GRP + GRP, N_BLK)))
                acc = {b: accp1.tile([128, F_IN], F32, tag="acc1",
                                     name=f"acc{g}_{b}") for b in blocks}
                for q in range(4):
                    seg = sch1["schedule"][g * 4 + q]
                    if seg["cols"] == 0:
                        continue
                    col_off = 0
                    col_base = seg["_slot_col0"]
                    for ncols in seg["calls"]:
                        w0 = (col_base + col_off) * 8
                        g_tile = gather_call(xt_t[QN[q]:QN[q + 1]], idx1_sb, w0, ncols)
                        J = sum(len(seg["jobs"][col_off + k]) for k in range(ncols))
                        mask_tile = mask_call(dst1_sb, job_i[0], J)
                        jj = 0
                        for k in range(ncols):
                            for b in seg["jobs"][col_off + k]:
                                st, sp = flags1[job_i[0]]
                                nc.tensor.matmul(
                                    acc[b],
                                    lhsT=mask_tile[:, jj * 128:(jj + 1) * 128],
                                    rhs=g_tile[:, k * 128 : k * 128 + F_IN],
                                    start=st, stop=sp,
                                )
                                jj += 1
                                job_i[0] += 1
                        col_off += ncols
                for b in blocks:
                    epi1(b, acc[b][:])
                for j in range(4):
                    if g + 1 == GQ[j + 1]:
                        nc.gpsimd.collective_compute(
                            "AllGather", ALU.bypass,
                            replica_groups=[list(range(NC))],
                            ins=[cc_in[R[j]:R[j + 1], :].opt()],
                            outs=[cc_outs[j][:, :].opt()],
                        )

            # ---------------- conv2 ----------------
            # Per (chunk, block): a short-lived psum accumulator (own bank,
            # from a 2-buf pool) drained into sacc2 in SBUF on its stop job.
            flags2 = sch2["flags"]
            runs_left = [0] * N_BLK
            ji = 0
            for seg in sch2["schedule"]:
                for col in range(seg["cols"]):
                    for b in seg["jobs"][col]:
                        if flags2[ji][1]:
                            runs_left[b] += 1
                        ji += 1

            def epi2(b):
                r = rows_of(b)
                zs = sbp.tile([128, 16], F32, tag="zs2")
                nc.vector.tensor_tensor(out=zs[:], in0=sacc2[:, b * 16:(b + 1) * 16],
                                        in1=own_g[:, b * 16:(b + 1) * 16], op=ALU.add)
                o1 = sbp.tile([128, 16], F32, tag="o1")
                nc.scalar.activation(o1[:], zs[:], AF.Copy, scale=dinv_sb[:, b : b + 1])
                o2 = sbp.tile([128, 16], F32, tag="o2")
                nc.vector.tensor_tensor(out=o2[:], in0=o1[:], in1=bf_sb[:], op=ALU.add)
                nc.sync.dma_start(out=out_t[b * BLK : b * BLK + r, :], in_=o2[:r, :])

            job_i2 = [0]
            open_acc = {}
            drained = set()
            for q in range(4):
                seg = sch2["schedule"][q]
                if seg["cols"] == 0:
                    continue
                col_off = 0
                col_base = seg["_slot_col0"]
                for ncols in seg["calls"]:
                    w0 = (col_base + col_off) * 8
                    g_tile = gather_call(cc_outs[q][:], idx2_sb, w0, ncols)
                    J = sum(len(seg["jobs"][col_off + k]) for k in range(ncols))
                    mask_tile = mask_call(dst2_sb, job_i2[0], J)
                    jj = 0
                    for k in range(ncols):
                        for b in seg["jobs"][col_off + k]:
                            st, sp = flags2[job_i2[0]]
                            if st:
                                assert b not in open_acc
                                open_acc[b] = accp2.tile([128, 16], F32, tag="acc2",
                                                         name=f"a2_{q}_{b}")
                            nc.tensor.matmul(
                                open_acc[b][:],
                                lhsT=mask_tile[:, jj * 128:(jj + 1) * 128],
                                rhs=g_tile[:, k * 128 : k * 128 + 16],
                                start=st, stop=sp,
                            )
                            if sp:
                                sl = sacc2[:, b * 16:(b + 1) * 16]
                                if b in drained:
                                    nc.vector.tensor_tensor(
                                        out=sl, in0=open_acc[b][:], in1=sl, op=ALU.add)
                                else:
                                    nc.scalar.activation(sl, open_acc[b][:], AF.Copy)
                                    drained.add(b)
                                del open_acc[b]
                                runs_left[b] -= 1
                                if runs_left[b] == 0:
                                    epi2(b)
                            jj += 1
                            job_i2[0] += 1
                    col_off += ncols
            assert not open_acc
            assert len(drained) == N_BLK
            assert all(v == 0 for v in runs_left)

    nc.compile()
    return nc


_CACHE = {}


def _in_maps(prep, dims, x, W1, b1, W2, b2, WL, bL):
    N, NC, N_LOC, BLK, N_BLK = (dims["N"], dims["NC"], dims["N_LOC"],
                                dims["BLK"], dims["N_BLK"])
    F_IN = dims["F_IN"]
    dinv = (1.0 / np.sqrt(prep["deg"])).astype(np.float32)
    xf = (np.asarray(x, np.float32) * dinv[:, None]).astype(np.float32)
    xt = np.zeros((N, 128), ml_dtypes.bfloat16)
    xt[:, :16] = xf.astype(ml_dtypes.bfloat16)
    Wf = (np.asarray(W2, np.float32) @ np.asarray(WL, np.float32)).astype(np.float32)
    bf = (np.asarray(b2, np.float32) @ np.asarray(WL, np.float32)
          + np.asarray(bL, np.float32)).astype(np.float32)
    w1b = np.concatenate([np.asarray(W1, np.float32),
                          np.asarray(b1, np.float32)[None, :]]).astype(np.float32)
    JMAX = prep["jmax"]
    iota = np.tile(np.arange(128, dtype=np.float32)[None, :],
                   (128, JMAX)).astype(ml_dtypes.bfloat16)
    ident = np.eye(128, dtype=np.float32)
    bft = np.tile(bf[None, :], (128, 1)).astype(np.float32)

    maps = []
    for c in range(NC):
        own_nodes = np.arange(c, N, NC)            # node of local row k = 8k+c
        # dinv per dst row, block-major partition layout [128, N_BLK]
        dv = np.ones(N_BLK * BLK, np.float32)
        dv[:N_LOC] = dinv[own_nodes]
        dinv_blk = np.ascontiguousarray(dv.reshape(N_BLK, BLK).T)
        # own x~ rows partition-major [128, N_BLK*F_IN]
        ox = np.zeros((N_BLK * BLK, F_IN), np.float32)
        ox[:N_LOC] = xf[own_nodes]
        ownx = np.ascontiguousarray(
            ox.reshape(N_BLK, BLK, F_IN).transpose(1, 0, 2).reshape(128, N_BLK * F_IN))
        maps.append(dict(
            xt=xt,
            idx1=prep["sch1"]["per_core"][c]["idx_w"],
            dst1=prep["sch1"]["per_core"][c]["dstloc"].astype(ml_dtypes.bfloat16),
            idx2=prep["sch2"]["per_core"][c]["idx_w"],
            dst2=prep["sch2"]["per_core"][c]["dstloc"].astype(ml_dtypes.bfloat16),
            dinv_blk=dinv_blk, ownx=ownx,
            w1b=w1b, wf=Wf, bft=bft, iota=iota, ident=ident,
            ones=np.ones((1, 128), np.float32),
        ))
    return maps


def kernel(**inputs):
    x = np.asarray(inputs["x"], np.float32)
    edge_index = np.asarray(inputs["edge_index"])
    W1 = np.asarray(inputs["W1"], np.float32)
    b1 = np.asarray(inputs["b1"], np.float32)
    W2 = np.asarray(inputs["W2"], np.float32)
    b2 = np.asarray(inputs["b2"], np.float32)
    WL = np.asarray(inputs["WL"], np.float32)
    bL = np.asarray(inputs["bL"], np.float32)

    if "nc" not in _CACHE:
        dims = make_dims(N=x.shape[0])
        prep = preprocess(edge_index.astype(np.int64), dims)
        nc = build(prep, dims)
        _CACHE.update(nc=nc, prep=prep, dims=dims)
    nc, prep, dims = _CACHE["nc"], _CACHE["prep"], _CACHE["dims"]

    maps = _in_maps(prep, dims, x, W1, b1, W2, b2, WL, bL)
    res = bass_utils.run_bass_kernel_spmd(nc, maps, core_ids=list(range(dims["NC"])))
    N, NC = dims["N"], dims["NC"]
    out = np.empty((N, 16), np.float32)
    for c in range(NC):
        out[c::NC] = res.results[c]["out"]
    return out.astype(np.float32)


# revision 29
# speedup vs baseline: 3.0228x; 3.0228x over previous
"""Trainium2 Bass kernel for a 2-layer GCN (nn_Net_49065706389774) — v2.

out = (S relu(S x W1 + b1)) (W2 WL) + (b2 WL + bL),  S = D^-1/2 (A+I) D^-1/2

v2 changes vs baseline:
 - node ownership n % 8 == c (so quarters of local rows = contiguous global
   node ranges) enabling 4 pipelined AllGathers overlapped with conv compute;
 - conv2 aggregates g = relu(h) @ (W2 WL) (16 features) instead of h (64);
 - conv2 is chunk-major with all 98 block accumulators live in hand-packed
   PSUM banks; epilogue2 is matmul-free;
 - epilogue1 computes h transposed (one PE transpose) and reuses relu(h).T
   as lhsT for the Wf matmul.
"""
import numpy as np
import ml_dtypes

import concourse.bass as bass
import concourse.bacc as bacc
import concourse.mybir as mybir
import concourse.tile as tile
from concourse import bass_utils

F32 = mybir.dt.float32
BF16 = mybir.dt.bfloat16
I16 = mybir.dt.int16
AF = mybir.ActivationFunctionType
ALU = mybir.AluOpType


def make_dims(N=100000):
    NC = 8
    N_LOC = N // NC          # 12500
    BLK = 128
    GRP = 4
    N_BLK = (N_LOC + BLK - 1) // BLK      # 98
    N_GRP = (N_BLK + GRP - 1) // GRP      # 25
    GQ = [0, 8, 14, 20, 25]               # group bounds per quarter
    RB = [min(g * GRP, N_BLK) for g in GQ]            # block bounds [0,30,54,78,98]
    R = [min(rb * BLK, N_LOC) for rb in RB]           # local row bounds
    QN = [NC * r for r in R]                          # global node bounds
    SQ = [R[j + 1] - R[j] for j in range(4)]
    return dict(N=N, NC=NC, N_LOC=N_LOC, BLK=BLK, GRP=GRP, N_BLK=N_BLK,
                N_GRP=N_GRP, GQ=GQ, RB=RB, R=R, QN=QN, SQ=SQ,
                F_IN=16, H1=64, COLS_PER_CALL=8)


def _make_schedule(core_segs, NC, N_BLK, COLS_PER_CALL, seg_group=None,
                   grp=4, per_segment_flags=False):
    """core_segs[sg][c] = (idx16 int64 array, kd int64 array) sorted by kd.

    Returns schedule + per-core idx/dstloc streams + flags.
    seg_group: optional list mapping segment -> dst group (for conv1 dummy-job
    placement which must stay within the block's group); None = any segment.
    per_segment_flags: start/stop accumulation scopes are per segment (conv2's
    short-lived psum accumulators) instead of global.
    """
    BLK = 128
    n_seg = len(core_segs)
    seg_cols = []
    for sg in range(n_seg):
        mx = max(len(core_segs[sg][c][0]) for c in range(NC))
        seg_cols.append((mx + BLK - 1) // BLK)

    schedule = []
    core_idx16 = [[] for _ in range(NC)]
    core_dst_of_slot = [[] for _ in range(NC)]
    core_blk_of_slot = [[] for _ in range(NC)]
    for sg in range(n_seg):
        C = seg_cols[sg]
        if C == 0:
            schedule.append(dict(sg=sg, cols=0, calls=[], jobs=[]))
            continue
        nslots = C * BLK
        col_jobs = [set() for _ in range(C)]
        for c in range(NC):
            iv, kdv = core_segs[sg][c]
            k = len(iv)
            i16 = np.zeros(nslots, np.int16)
            i16[:k] = iv.astype(np.int16)
            dl = np.full(nslots, -1, np.int32)
            dl[:k] = kdv
            bl = np.full(nslots, -1, np.int32)
            bl[:k] = kdv // BLK
            core_idx16[c].append(i16)
            core_dst_of_slot[c].append(dl)
            core_blk_of_slot[c].append(bl)
            for col in range(C):
                for b in np.unique(bl[col * BLK:(col + 1) * BLK]):
                    if b >= 0:
                        col_jobs[col].add(int(b))
        prev = None
        for col in range(C):
            if not col_jobs[col]:
                fallback = prev if prev is not None else (
                    seg_group[sg] * grp if seg_group is not None else 0)
                col_jobs[col] = {fallback}
            prev = max(col_jobs[col])
        calls = []
        off = 0
        while off < C:
            calls.append(min(COLS_PER_CALL, C - off))
            off += COLS_PER_CALL
        schedule.append(dict(sg=sg, cols=C, calls=calls,
                             jobs=[sorted(col_jobs[col]) for col in range(C)]))

    # blocks with zero jobs -> inject dummy job so psum gets start/stop
    jobs_per_block = np.zeros(N_BLK, np.int64)
    for seg in schedule:
        for jl in seg["jobs"]:
            for b in jl:
                jobs_per_block[b] += 1
    for b in range(N_BLK):
        if jobs_per_block[b] == 0:
            placed = False
            for si, seg in enumerate(schedule):
                if seg["cols"] == 0:
                    continue
                if seg_group is not None and seg_group[si] != b // grp:
                    continue
                seg["jobs"][0] = sorted(set(seg["jobs"][0]) | {b})
                jobs_per_block[b] += 1
                placed = True
                break
            assert placed, f"no segment for dummy job of block {b}"

    # start/stop flags in traversal order
    flags = []
    if per_segment_flags:
        # Accumulation scopes are per (block, run of consecutive columns)
        # within a segment: short-lived psum tiles drained at each run end.
        for seg in schedule:
            occ = {}
            for col in range(seg["cols"]):
                for b in seg["jobs"][col]:
                    occ.setdefault(b, []).append(col)
            rs, re = {}, {}
            for b, cols in occ.items():
                for i, col in enumerate(cols):
                    rs[(b, col)] = (i == 0) or (cols[i - 1] != col - 1)
                    re[(b, col)] = (i == len(cols) - 1) or (cols[i + 1] != col + 1)
            for col in range(seg["cols"]):
                for b in seg["jobs"][col]:
                    flags.append((rs[(b, col)], re[(b, col)]))
        # pool-size check: max concurrently-open accumulators in emission order
        open_b, max_open = set(), 0
        ji = 0
        for seg in schedule:
            for col in range(seg["cols"]):
                for b in seg["jobs"][col]:
                    st, sp = flags[ji]
                    if st:
                        open_b.add(b)
                        max_open = max(max_open, len(open_b))
                    if sp:
                        open_b.discard(b)
                    ji += 1
        assert max_open <= 3, f"psum acc pool too small: {max_open} open"
        n_jobs = len(flags)
    else:
        first_seen, last_seen = {}, {}
        ji = 0
        for seg in schedule:
            for col in range(seg["cols"]):
                for b in seg["jobs"][col]:
                    if b not in first_seen:
                        first_seen[b] = ji
                    last_seen[b] = ji
                    ji += 1
        n_jobs = ji
        ji = 0
        for seg in schedule:
            for col in range(seg["cols"]):
                for b in seg["jobs"][col]:
                    flags.append((ji == first_seen[b], ji == last_seen[b]))
                    ji += 1

    # per-core streams
    per_core = []
    for c in range(NC):
        idx16 = (np.concatenate(core_idx16[c]) if core_idx16[c]
                 else np.zeros(0, np.int16))
        S = len(idx16)
        assert S % 16 == 0
        idx_w = np.tile(idx16.reshape(S // 16, 16).T, (8, 1))  # [128, S/16]
        dstlocs = []
        seg_i = 0
        for seg in schedule:
            if seg["cols"] == 0:
                continue
            dl = core_dst_of_slot[c][seg_i]
            bl = core_blk_of_slot[c][seg_i]
            for col in range(seg["cols"]):
                dcol = dl[col * BLK:(col + 1) * BLK]
                bcol = bl[col * BLK:(col + 1) * BLK]
                for b in seg["jobs"][col]:
                    rel = np.where(bcol == b, dcol - b * BLK, -1).astype(np.float32)
                    dstlocs.append(rel)
            seg_i += 1
        dstloc = np.stack(dstlocs, axis=1)  # [128, n_jobs]
        assert dstloc.shape[1] == n_jobs
        per_core.append(dict(idx_w=idx_w, dstloc=dstloc))

    max_jobs_per_call = 0
    for seg in schedule:
        off = 0
        for ncols in seg["calls"]:
            j = sum(len(seg["jobs"][off + k]) for k in range(ncols))
            max_jobs_per_call = max(max_jobs_per_call, j)
            off += ncols

    n_slots = sum(s["cols"] for s in schedule) * BLK
    # slot column offsets for gather idx addressing
    sc = 0
    for seg in schedule:
        seg["_slot_col0"] = sc
        sc += seg["cols"]
    return dict(schedule=schedule, per_core=per_core, flags=flags,
                n_jobs=n_jobs, n_slots=n_slots,
                max_jobs_per_call=max_jobs_per_call)


def preprocess(edge_index, dims):
    N, NC, BLK, GRP = dims["N"], dims["NC"], dims["BLK"], dims["GRP"]
    N_BLK, N_GRP = dims["N_BLK"], dims["N_GRP"]
    R, QN, SQ = dims["R"], dims["QN"], dims["SQ"]
    src = np.asarray(edge_index[0], np.int64)
    dst = np.asarray(edge_index[1], np.int64)
    deg = (np.bincount(dst, minlength=N) + 1.0).astype(np.float32)

    QNa = np.asarray(QN, np.int64)
    segs1 = [[None] * NC for _ in range(N_GRP * 4)]
    segs2 = [[None] * NC for _ in range(4)]
    for c in range(NC):
        m = (dst % NC) == c
        s = src[m]
        kd = dst[m] // NC
        qk = np.searchsorted(QNa, s, side="right") - 1
        # conv1: by (group, chunk, dst)
        gk = kd // (GRP * BLK)
        o1 = np.lexsort((kd, qk, gk))
        s1, kd1, q1, g1 = s[o1], kd[o1], qk[o1], gk[o1]
        for g in range(N_GRP):
            for q in range(4):
                mm = (g1 == g) & (q1 == q)
                segs1[g * 4 + q][c] = (s1[mm] - QN[q], kd1[mm])
        # conv2: by (chunk, dst)
        o2 = np.lexsort((kd, qk))
        s2, kd2, q2 = s[o2], kd[o2], qk[o2]
        for q in range(4):
            mm = q2 == q
            sq = s2[mm]
            idx2 = (sq % NC) * SQ[q] + (sq // NC - R[q])
            segs2[q][c] = (idx2, kd2[mm])

    seg_group1 = [sg // 4 for sg in range(N_GRP * 4)]
    sch1 = _make_schedule(segs1, NC, N_BLK, dims["COLS_PER_CALL"], seg_group1,
                          grp=GRP)
    sch2 = _make_schedule(segs2, NC, N_BLK, dims["COLS_PER_CALL"], None,
                          grp=GRP, per_segment_flags=True)
    return dict(deg=deg, sch1=sch1, sch2=sch2,
                jmax=max(sch1["max_jobs_per_call"], sch2["max_jobs_per_call"]))


def build(prep, dims):
    N, NC, N_LOC, BLK = dims["N"], dims["NC"], dims["N_LOC"], dims["BLK"]
    GRP, N_BLK, N_GRP = dims["GRP"], dims["N_BLK"], dims["N_GRP"]
    GQ, R, QN, SQ = dims["GQ"], dims["R"], dims["QN"], dims["SQ"]
    F_IN, H1 = dims["F_IN"], dims["H1"]
    JMAX = prep["jmax"]
    sch1, sch2 = prep["sch1"], prep["sch2"]

    S1 = sch1["per_core"][0]["idx_w"].shape[1]
    S2 = sch2["per_core"][0]["idx_w"].shape[1]
    J1 = sch1["n_jobs"]
    J2 = sch2["n_jobs"]

    nc = bacc.Bacc("TRN2", target_bir_lowering=False, debug=False,
                   num_devices=NC, num_swdge_queues=4)
    xt_t = nc.dram_tensor("xt", [N, 128], BF16, kind="ExternalInput").ap()
    idx1_t = nc.dram_tensor("idx1", [128, S1], I16, kind="ExternalInput").ap()
    dst1_t = nc.dram_tensor("dst1", [128, J1], BF16, kind="ExternalInput").ap()
    idx2_t = nc.dram_tensor("idx2", [128, S2], I16, kind="ExternalInput").ap()
    dst2_t = nc.dram_tensor("dst2", [128, J2], BF16, kind="ExternalInput").ap()
    dinv_t = nc.dram_tensor("dinv_blk", [128, N_BLK], F32, kind="ExternalInput").ap()
    ownx_t = nc.dram_tensor("ownx", [128, N_BLK * F_IN], F32, kind="ExternalInput").ap()
    w1b_t = nc.dram_tensor("w1b", [F_IN + 1, H1], F32, kind="ExternalInput").ap()
    wf_t = nc.dram_tensor("wf", [H1, 16], F32, kind="ExternalInput").ap()
    bf_t = nc.dram_tensor("bft", [128, 16], F32, kind="ExternalInput").ap()
    iota_t = nc.dram_tensor("iota", [128, JMAX * 128], BF16, kind="ExternalInput").ap()
    ident_t = nc.dram_tensor("ident", [128, 128], F32, kind="ExternalInput").ap()
    ones_t = nc.dram_tensor("ones", [1, 128], F32, kind="ExternalInput").ap()
    out_t = nc.dram_tensor("out", [N_LOC, 16], F32, kind="ExternalOutput").ap()

    with tile.TileContext(nc) as tc:
        with (
            tc.tile_pool(name="const", bufs=1) as constp,
            tc.tile_pool(name="dram", bufs=1, space="DRAM") as dramp,
            tc.tile_pool(name="gat", bufs=8) as gatp,
            tc.tile_pool(name="msk", bufs=4) as mskp,
            tc.tile_pool(name="acc1", bufs=4, space="PSUM") as accp1,
            tc.tile_pool(name="acc2", bufs=3, space="PSUM") as accp2,
            tc.tile_pool(name="pb", bufs=1, space="PSUM") as pbank,
            tc.tile_pool(name="sb", bufs=4) as sbp,
        ):
            idx1_sb = constp.tile([128, S1], I16)
            dst1_sb = constp.tile([128, J1], BF16)
            idx2_sb = constp.tile([128, S2], I16)
            dst2_sb = constp.tile([128, J2], BF16)
            for tsb, tdr, W in ((idx1_sb, idx1_t, S1), (dst1_sb, dst1_t, J1),
                                (idx2_sb, idx2_t, S2), (dst2_sb, dst2_t, J2)):
                step = (W + 3) // 4
                for o in range(0, W, step):
                    e = min(o + step, W)
                    nc.sync.dma_start(out=tsb[:, o:e], in_=tdr[:, o:e])
            dinv_sb = constp.tile([128, N_BLK], F32)
            nc.sync.dma_start(out=dinv_sb[:], in_=dinv_t[:])
            ownx_sb = constp.tile([128, N_BLK * F_IN], F32)
            nc.sync.dma_start(out=ownx_sb[:], in_=ownx_t[:])
            w1b_sb = constp.tile([F_IN + 1, H1], F32)
            nc.sync.dma_start(out=w1b_sb[:], in_=w1b_t[:])
            wf_sb = constp.tile([H1, 16], F32)
            nc.sync.dma_start(out=wf_sb[:], in_=wf_t[:])
            bf_sb = constp.tile([128, 16], F32)
            nc.sync.dma_start(out=bf_sb[:], in_=bf_t[:])
            iota_sb = constp.tile([128, JMAX * 128], BF16)
            nc.sync.dma_start(out=iota_sb[:], in_=iota_t[:])
            ident_sb = constp.tile([128, 128], F32)
            nc.sync.dma_start(out=ident_sb[:], in_=ident_t[:])
            own_g = constp.tile([128, N_BLK * 16], BF16)
            z1T_tiles = [constp.tile([F_IN + 1, 128], F32, name=f"z1T_{i}")
                         for i in range(2)]
            for t in z1T_tiles:
                nc.sync.dma_start(out=t[F_IN : F_IN + 1, :], in_=ones_t[:])

            cc_in = dramp.tile([N_LOC, 128], BF16)
            cc_outs = [dramp.tile([NC * SQ[j], 128], BF16, addr_space="Shared",
                                  name=f"cc_out{j}") for j in range(4)]

            # PSUM: conv1 acc pool 4 banks, conv2 acc pool 3 banks, epi1
            # 1 packed bank (tp/hpT/g share it; start=True clears a whole
            # bank's has_written, so each live accumulator owns a bank).
            epib = [pbank.tile([128, 272], F32, name="epi_0")]
            # conv2 block partial sums across chunks accumulate in SBUF
            sacc2 = constp.tile([128, N_BLK * 16], F32)

            gq = [0]

            def gather_call(table_ap, idx_sb, w0, ncols):
                nidx = ncols * BLK
                g_tile = gatp.tile([128, 8 * 128], BF16, tag="gat")
                nc.gpsimd.dma_gather(
                    out_ap=g_tile[:, : ncols * 128].rearrange("p (c e) -> p c e", e=128),
                    in_ap=table_ap,
                    idxs_ap=idx_sb[:, w0 : w0 + ncols * 8],
                    num_idxs=nidx,
                    num_idxs_reg=nidx,
                    elem_size=128,
                    queue_num=gq[0] % 4,
                )
                gq[0] += 1
                return g_tile

            def mask_call(dst_sb, j0, J):
                mask_tile = mskp.tile([128, JMAX * 128], BF16, tag="msk")
                nc.vector.tensor_tensor(
                    out=mask_tile[:, : J * 128],
                    in0=dst_sb[:, j0 : j0 + J].to_broadcast((128, J, 128)),
                    in1=iota_sb[:, : J * 128],
                    op=ALU.is_equal,
                )
                return mask_tile

            def rows_of(b):
                return min(BLK, N_LOC - b * BLK)

            def epi1(b, acc_ap):
                r = rows_of(b)
                eb = epib[0]
                tp = eb[0:F_IN, 0:128]
                hpT = eb[0:H1, 128:256]
                gps = eb[:, 256:272]
                zs = sbp.tile([128, F_IN], F32, tag="zs1")
                nc.vector.tensor_tensor(out=zs[:], in0=acc_ap,
                                        in1=ownx_sb[:, b * F_IN:(b + 1) * F_IN],
                                        op=ALU.add)
                z1 = sbp.tile([128, F_IN], F32, tag="z1")
                nc.scalar.activation(z1[:], zs[:], AF.Copy, scale=dinv_sb[:, b : b + 1])
                nc.tensor.transpose(tp, z1[:], ident_sb[:])
                z1T = z1T_tiles[b % 2]
                nc.scalar.activation(z1T[:F_IN, :], tp, AF.Copy)
                nc.tensor.matmul(hpT, lhsT=w1b_sb[:], rhs=z1T[:], start=True, stop=True)
                rhT = sbp.tile([H1, 128], F32, tag="rhT")
                nc.scalar.activation(rhT[:], hpT, AF.Relu)
                nc.tensor.matmul(gps, lhsT=rhT[:], rhs=wf_sb[:], start=True, stop=True)
                gp = sbp.tile([128, 128], BF16, tag="gp")
                nc.scalar.activation(gp[:, 0:16], gps, AF.Copy, scale=dinv_sb[:, b : b + 1])
                nc.scalar.activation(own_g[:, b * 16:(b + 1) * 16], gp[:, 0:16], AF.Copy)
                nc.sync.dma_start(out=cc_in[b * BLK : b * BLK + r, :], in_=gp[:r, :])

            # ---------------- conv1 ----------------
            flags1 = sch1["flags"]
            job_i = [0]
            for g in range(N_GRP):
                blocks = list(range(g * GRP, min(g * GRP + GRP, N_BLK)))
                acc = {b: accp1.tile([128, F_IN], F32, tag="acc1",
                                     name=f"acc{g}_{b}") for b in blocks}
                for q in range(4):
                    seg = sch1["schedule"][g * 4 + q]
                    if seg["cols"] == 0:
                        continue
                    col_off = 0
                    col_base = seg["_slot_col0"]
                    for ncols in seg["calls"]:
                        w0 = (col_base + col_off) * 8
                        g_tile = gather_call(xt_t[QN[q]:QN[q + 1]], idx1_sb, w0, ncols)
                        J = sum(len(seg["jobs"][col_off + k]) for k in range(ncols))
                        mask_tile = mask_call(dst1_sb, job_i[0], J)
                        jj = 0
                        for k in range(ncols):
                            for b in seg["jobs"][col_off + k]:
                                st, sp = flags1[job_i[0]]
                                nc.tensor.matmul(
                                    acc[b],
                                    lhsT=mask_tile[:, jj * 128:(jj + 1) * 128],
                                    rhs=g_tile[:, k * 128 : k * 128 + F_IN],
                                    start=st, stop=sp,
                                )
                                jj += 1
                                job_i[0] += 1
                        col_off += ncols
                for b in blocks:
                    epi1(b, acc[b][:])
                for j in range(4):
                    if g + 1 == GQ[j + 1]:
                        nc.gpsimd.collective_compute(
                            "AllGather", ALU.bypass,
                            replica_groups=[list(range(NC))],
                            ins=[cc_in[R[j]:R[j + 1], :].opt()],
                            outs=[cc_outs[j][:, :].opt()],
                        )

            # ---------------- conv2 ----------------
            # Per (chunk, block): a short-lived psum accumulator (own bank,
            # from a 2-buf pool) drained into sacc2 in SBUF on its stop job.
            flags2 = sch2["flags"]
            runs_left = [0] * N_BLK
            ji = 0
            for seg in sch2["schedule"]:
                for col in range(seg["cols"]):
                    for b in seg["jobs"][col]:
                        if flags2[ji][1]:
                            runs_left[b] += 1
                        ji += 1

            def epi2(b):
                r = rows_of(b)
                zs = sbp.tile([128, 16], F32, tag="zs2")
                nc.vector.tensor_tensor(out=zs[:], in0=sacc2[:, b * 16:(b + 1) * 16],
                                        in1=own_g[:, b * 16:(b + 1) * 16], op=ALU.add)
                o1 = sbp.tile([128, 16], F32, tag="o1")
                nc.scalar.activation(o1[:], zs[:], AF.Copy, scale=dinv_sb[:, b : b + 1])
                o2 = sbp.tile([128, 16], F32, tag="o2")
                nc.vector.tensor_tensor(out=o2[:], in0=o1[:], in1=bf_sb[:], op=ALU.add)
                nc.sync.dma_start(out=out_t[b * BLK : b * BLK + r, :], in_=o2[:r, :])

            job_i2 = [0]
            open_acc = {}
            drained = set()
            for q in range(4):
                seg = sch2["schedule"][q]
                if seg["cols"] == 0:
                    continue
                col_off = 0
                col_base = seg["_slot_col0"]
                for ncols in seg["calls"]:
                    w0 = (col_base + col_off) * 8
                    g_tile = gather_call(cc_outs[q][:], idx2_sb, w0, ncols)
                    J = sum(len(seg["jobs"][col_off + k]) for k in range(ncols))
                    mask_tile = mask_call(dst2_sb, job_i2[0], J)
                    jj = 0
                    for k in range(ncols):
                        for b in seg["jobs"][col_off + k]:
                            st, sp = flags2[job_i2[0]]
                            if st:
                                assert b not in open_acc
                                open_acc[b] = accp2.tile([128, 16], F32, tag="acc2",
                                                         name=f"a2_{q}_{b}")
                            nc.tensor.matmul(
                                open_acc[b][:],
                                lhsT=mask_tile[:, jj * 128:(jj + 1) * 128],
                                rhs=g_tile[:, k * 128 : k * 128 + 16],
                                start=st, stop=sp,
                            )
                            if sp:
                                sl = sacc2[:, b * 16:(b + 1) * 16]
                                if b in drained:
                                    nc.vector.tensor_tensor(
                                        out=sl, in0=open_acc[b][:], in1=sl, op=ALU.add)
                                else:
                                    nc.scalar.activation(sl, open_acc[b][:], AF.Copy)
                                    drained.add(b)
                                del open_acc[b]
                                runs_left[b] -= 1
                                if runs_left[b] == 0:
                                    epi2(b)
                            jj += 1
                            job_i2[0] += 1
                    col_off += ncols
            assert not open_acc
            assert len(drained) == N_BLK
            assert all(v == 0 for v in runs_left)

    nc.compile()
    return nc


_CACHE = {}


def _in_maps(prep, dims, x, W1, b1, W2, b2, WL, bL):
    N, NC, N_LOC, BLK, N_BLK = (dims["N"], dims["NC"], dims["N_LOC"],
                                dims["BLK"], dims["N_BLK"])
    F_IN = dims["F_IN"]
    dinv = (1.0 / np.sqrt(prep["deg"])).astype(np.float32)
    xf = (np.asarray(x, np.float32) * dinv[:, None]).astype(np.float32)
    xt = np.zeros((N, 128), ml_dtypes.bfloat16)
    xt[:, :16] = xf.astype(ml_dtypes.bfloat16)
    Wf = (np.asarray(W2, np.float32) @ np.asarray(WL, np.float32)).astype(np.float32)
    bf = (np.asarray(b2, np.float32) @ np.asarray(WL, np.float32)
          + np.asarray(bL, np.float32)).astype(np.float32)
    w1b = np.concatenate([np.asarray(W1, np.float32),
                          np.asarray(b1, np.float32)[None, :]]).astype(np.float32)
    JMAX = prep["jmax"]
    iota = np.tile(np.arange(128, dtype=np.float32)[None, :],
                   (128, JMAX)).astype(ml_dtypes.bfloat16)
    ident = np.eye(128, dtype=np.float32)
    bft = np.tile(bf[None, :], (128, 1)).astype(np.float32)

    maps = []
    for c in range(NC):
        own_nodes = np.arange(c, N, NC)            # node of local row k = 8k+c
        # dinv per dst row, block-major partition layout [128, N_BLK]
        dv = np.ones(N_BLK * BLK, np.float32)
        dv[:N_LOC] = dinv[own_nodes]
        dinv_blk = np.ascontiguousarray(dv.reshape(N_BLK, BLK).T)
        # own x~ rows partition-major [128, N_BLK*F_IN]
        ox = np.zeros((N_BLK * BLK, F_IN), np.float32)
        ox[:N_LOC] = xf[own_nodes]
        ownx = np.ascontiguousarray(
            ox.reshape(N_BLK, BLK, F_IN).transpose(1, 0, 2).reshape(128, N_BLK * F_IN))
        maps.append(dict(
            xt=xt,
            idx1=prep["sch1"]["per_core"][c]["idx_w"],
            dst1=prep["sch1"]["per_core"][c]["dstloc"].astype(ml_dtypes.bfloat16),
            idx2=prep["sch2"]["per_core"][c]["idx_w"],
            dst2=prep["sch2"]["per_core"][c]["dstloc"].astype(ml_dtypes.bfloat16),
            dinv_blk=dinv_blk, ownx=ownx,
            w1b=w1b, wf=Wf, bft=bft, iota=iota, ident=ident,
            ones=np.ones((1, 128), np.float32),
        ))
    return maps


def kernel(**inputs):
    x = np.asarray(inputs["x"], np.float32)
    edge_index = np.asarray(inputs["edge_index"])
    W1 = np.asarray(inputs["W1"], np.float32)
    b1 = np.asarray(inputs["b1"], np.float32)
    W2 = np.asarray(inputs["W2"], np.float32)
    b2 = np.asarray(inputs["b2"], np.float32)
    WL = np.asarray(inputs["WL"], np.float32)
    bL = np.asarray(inputs["bL"], np.float32)

    if "nc" not in _CACHE:
        dims = make_dims(N=x.shape[0])
        prep = preprocess(edge_index.astype(np.int64), dims)
        nc = build(prep, dims)
        _CACHE.update(nc=nc, prep=prep, dims=dims)
    nc, prep, dims = _CACHE["nc"], _CACHE["prep"], _CACHE["dims"]

    maps = _in_maps(prep, dims, x, W1, b1, W2, b2, WL, bL)
    res = bass_utils.run_bass_kernel_spmd(nc, maps, core_ids=list(range(dims["NC"])))
    N, NC = dims["N"], dims["NC"]
    out = np.empty((N, 16), np.float32)
    for c in range(NC):
        out[c::NC] = res.results[c]["out"]
    return out.astype(np.float32)
